# revision 6
# baseline (speedup 1.0000x reference)
"""Trainium2 Bass kernel for nn_MultiHeadAttention_84791244358011.

Linear (ELU feature-map) attention:
    x_norm = LayerNorm(x)                      # eps=1e-12
    q = x_norm @ Wq.T + bq ; k,v = x @ W.T + b # per-head [S, 64]
    eq/ek = l2norm(elu(q/k)) per token over head_dim
    kv = ek^T @ v per head [64, 64]; ctx = eq @ kv / 8
    out = ctx @ Wo.T + bo + x

Sharding: data-parallel over batch B=8 — one batch element per NeuronCore,
no collectives.

v2 design: LayerNorm is folded into the q projection algebraically:
    q = rstd * (x @ Wg^T) - (mu*rstd) * c + d
with Wg[i,j] = gamma[i]*Wq[j,i], c = column sums of Wg, d = bq + beta@Wq.T
(all host-precomputed). So q/k/v all project from the same transposed-x
stationary tiles in a SINGLE pass (no z tensor, no zT spill):
  per 128-token tile: LN stats -> rstd, s=mu*rstd; PE-transpose x (and the
  [s;1] column pair -> aug row); 8x interleaved matmuls accumulate q/k/v
  half-banks (q gets a K=2 aug matmul adding -s*c + d); elu+l2norm on q,k
  (rstd rides the ACT scale / DVE tensor_scalar ops); per-head kv state
  accumulates col-tiled into ONE psum bank (even heads -> partitions 0:64,
  odd heads -> 64:128); eq is PE-transposed and spilled to DRAM as bf16.
Pass B: per 512-token chunk, ctx^T = kv @ eq^T (col/row-tiled 64-K pairs
into one bank), out = ctx^T.T @ wot + x (+bo).

PSUM: tag "A" rotates s-transpose/x-transpose/q-halves/eq-transpose through
4 banks; tag "B" rotates k/v halves through 3; kv state holds 1.
"""

import numpy as np

import concourse.bass as bass
import concourse.mybir as mybir
import concourse.tile as tile
from concourse import bacc
from concourse.masks import make_identity

B, S, HID = 8, 4096, 1024
NH, HD = 16, 64
P = 128
NT = S // P            # 32 token tiles
NC = HID // P          # 8 feature chunks
HNH = NH // 2          # heads per half
CHUNK = 4              # token tiles per ctx chunk (512 tokens)
NCHUNKS = NT // CHUNK
LN_EPS = 1e-12

F32 = mybir.dt.float32
F32R = mybir.dt.float32r
BF16 = mybir.dt.bfloat16
I32 = mybir.dt.int32
AF = mybir.ActivationFunctionType
OP = mybir.AluOpType


def _rsqrt(nc, pool, consts, src, n, name, iters=2):
    """1/sqrt(src[:, :n]) on DVE: quake-III seed + Newton iterations."""
    magic_b, one_b = consts
    e = nc.vector
    shi = pool.tile([P, n], I32, tag=f"rq_sh{n}", bufs=4, name=f"{name}_shi")
    e.tensor_tensor(shi, src.bitcast(I32), one_b[:, 0:1].to_broadcast((P, n)),
                    OP.arith_shift_right)
    y0i = pool.tile([P, n], I32, tag=f"rq_y0{n}", bufs=4, name=f"{name}_y0i")
    e.tensor_tensor(y0i, magic_b[:, 0:1].to_broadcast((P, n)), shi, OP.subtract)
    h = pool.tile([P, n], F32, tag=f"rq_h{n}", bufs=4, name=f"{name}_h")
    e.tensor_scalar(h, src, -0.5, None, OP.mult)
    cur = y0i[:].bitcast(F32)
    for it in range(iters):
        t = pool.tile([P, n], F32, tag=f"rq_t{n}_{it}", bufs=4,
                      name=f"{name}_t{it}")
        e.tensor_tensor(t, cur, cur, OP.mult)
        e.tensor_tensor(t, t, h, OP.mult)
        e.tensor_scalar(t, t, 1.5, None, OP.add)
        y = pool.tile([P, n], F32, tag=f"rq_y{n}_{it}", bufs=4,
                      name=f"{name}_y{it}")
        e.tensor_tensor(y, cur, t, OP.mult)
        cur = y
    return cur


def build_nc(loop_n=1, bk_nz=False, bv_nz=False, bo_nz=False, debug=False):
    nc = bacc.Bacc("TRN2", target_bir_lowering=False, enable_partition_id=False)
    dbg = {}
    if debug:
        dbg["ek0"] = nc.dram_tensor("dbg_ek0", [P, HID], F32, kind="ExternalOutput")
        dbg["v0"] = nc.dram_tensor("dbg_v0", [P, HID], F32, kind="ExternalOutput")
        dbg["kv"] = nc.dram_tensor("dbg_kv", [HD, NH * HD], F32,
                                   kind="ExternalOutput")
        dbg["eq0"] = nc.dram_tensor("dbg_eq0", [P, HID], F32, kind="ExternalOutput")

    x_d = nc.dram_tensor("x", [S, HID], F32, kind="ExternalInput")
    wqt_d = nc.dram_tensor("wqt", [HID, HID], F32, kind="ExternalInput")
    wkt_d = nc.dram_tensor("wkt", [HID, HID], F32, kind="ExternalInput")
    wvt_d = nc.dram_tensor("wvt", [HID, HID], F32, kind="ExternalInput")
    wot_d = nc.dram_tensor("wot", [HID, HID], F32, kind="ExternalInput")
    cdq_d = nc.dram_tensor("cdq", [2, HID], F32, kind="ExternalInput")
    bk_d = nc.dram_tensor("bk", [1, HID], F32, kind="ExternalInput")
    bv_d = nc.dram_tensor("bv", [1, HID], F32, kind="ExternalInput")
    bo_d = nc.dram_tensor("bo", [1, HID], F32, kind="ExternalInput")
    out_d = nc.dram_tensor("out", [S, HID], F32, kind="ExternalOutput")
    eqt_d = nc.dram_tensor("eqt_spill", [NT, P, NC * P], BF16)

    import contextlib

    with tile.TileContext(nc) as tc, contextlib.ExitStack() as ctx:
        persist = ctx.enter_context(tc.tile_pool(name="persist", bufs=1))
        wpool = ctx.enter_context(tc.tile_pool(name="weights", bufs=1))

        ident = persist.tile([P, P], F32)
        make_identity(nc, ident)
        ident_r = persist.tile([P, P], F32R)
        nc.scalar.activation(ident_r, ident, AF.Copy)
        magic_t = persist.tile([P, 1], I32)
        nc.gpsimd.memset(magic_t, 0x5F3759DF)
        one_t = persist.tile([P, 1], I32)
        nc.gpsimd.memset(one_t, 1)
        consts = (magic_t, one_t)
        kv_sb = persist.tile([P, NC * HD], BF16)   # 2 heads per 128 partitions
        cdq_sb = persist.tile([2, HID], F32R, name="cdq_sb")
        nc.sync.dma_start(cdq_sb, cdq_d.ap().bitcast(F32R))
        brep = {}
        reps = []
        if bk_nz:
            reps.append(("bk", bk_d))
        if bv_nz:
            reps.append(("bv", bv_d))
        if bo_nz:
            reps.append(("bo", bo_d))
        for nm, d in reps:
            t_ = persist.tile([P, HID], F32, name=f"{nm}_rep")
            h = d.ap()
            nc.gpsimd.dma_start(
                t_, bass.AP(tensor=h.tensor, offset=h.offset,
                            ap=[[0, P], [1, HID]]))
            brep[nm] = t_

        _loop = tc.For_i(0, loop_n, 1) if loop_n > 1 else contextlib.nullcontext(0)
        with _loop:
            # weights (all four resident; f32r, contraction dim on partitions)
            w_sb = {}
            for nm, d in (("wq", wqt_d), ("wk", wkt_d), ("wv", wvt_d)):
                t_ = wpool.tile([P, NC, HID], F32R, tag=f"w_{nm}", name=f"{nm}_sb")
                nc.sync.dma_start(
                    t_, d.ap().rearrange("(c p) j -> p c j", p=P).bitcast(F32R))
                w_sb[nm] = t_

            # ---------------- pass A: q/k/v + kv state + eqT spill ---------
            with tc.tile_pool(name="sbufA", bufs=1) as sa, \
                 tc.tile_pool(name="psumA", bufs=1, space="PSUM") as pa:
                kv_ps = pa.tile([HD, NH * HD], F32, tag="kv", name="kv_ps")

                for t in range(NT):
                    x_t = sa.tile([P, HID], F32, tag="x", bufs=3, name=f"x_{t}")
                    nc.scalar.dma_start(x_t, x_d.ap()[t * P:(t + 1) * P, :])

                    # LayerNorm stats -> rstd, s = mu*rstd
                    stats = sa.tile([P, 2, 6], F32, tag="st", bufs=2,
                                    name=f"st_{t}")
                    xg = x_t[:].rearrange("p (g d) -> p g d", g=2)
                    for g in range(2):
                        nc.vector.bn_stats(stats[:, g, :], xg[:, g, :])
                    mv = sa.tile([P, 2], F32, tag="mv", bufs=4, name=f"mv_{t}")
                    nc.vector.bn_aggr(mv, stats)
                    vpe = sa.tile([P, 1], F32, tag="sd", bufs=4, name=f"sd_{t}")
                    nc.vector.tensor_scalar(vpe, mv[:, 1:2], LN_EPS, None, OP.add)
                    rstd = _rsqrt(nc, sa, consts, vpe[:], 1, f"rs_{t}")
                    nrstd = sa.tile([P, 1], F32, tag="nrs", bufs=4,
                                    name=f"nrs_{t}")
                    nc.vector.tensor_scalar(nrstd, rstd, -1.0, None, OP.mult)
                    # s2 = [mu, std] columns: psum gets -mu*c + std*d, which
                    # the later rstd scaling turns into -mu*rstd*c + d.
                    s2 = sa.tile([P, 2], F32R, tag="s2", bufs=4, name=f"s2_{t}")
                    nc.vector.tensor_copy(s2[:, 0:1], mv[:, 0:1])
                    nc.scalar.activation(s2[:, 1:2], vpe, AF.Sqrt)

                    # s-transpose -> augT [2, 128] (rows: mu^T, std^T)
                    stp = pa.tile([P, 4 * P], F32, tag="A", bufs=2,
                                  name=f"stp_{t}")
                    nc.tensor.transpose(
                        stp[0:2, 0:P].bitcast(F32R), s2[:], ident_r)
                    augT = sa.tile([2, P], F32R, tag="augT", bufs=4,
                                   name=f"augT_{t}")
                    nc.scalar.copy(augT[:], stp[0:2, 0:P])

                    # transpose x (PE) -> xT
                    xT = sa.tile([P, NC, P], F32R, tag="xT", bufs=2,
                                 name=f"xT_{t}")
                    for half in range(2):
                        tp = pa.tile([P, 4 * P], F32, tag="A", bufs=2,
                                     name=f"tp_{t}_{half}")
                        for b_ in range(4):
                            blk = half * 4 + b_
                            nc.tensor.transpose(
                                tp[:, b_ * P:(b_ + 1) * P],
                                x_t[:, blk * P:(blk + 1) * P], ident)
                        nc.scalar.copy(
                            xT[:].rearrange("p c s -> p (c s)")[
                                :, half * 4 * P:(half + 1) * 4 * P], tp)

                    # q/k/v projections, interleaved over the shared xT
                    # stationary. q into tag-A banks, k/v into tag-B banks.
                    q_ps = [pa.tile([P, 512], F32, tag="A", bufs=2,
                                    name=f"q_ps{t}_{h}") for h in range(2)]
                    k_ps = [pa.tile([P, 512], F32, tag="B", bufs=4,
                                    name=f"k_ps{t}_{h}") for h in range(2)]
                    v_ps = [pa.tile([P, 512], F32, tag="B", bufs=4,
                                    name=f"v_ps{t}_{h}") for h in range(2)]
                    for c in range(NC):
                        st = (c == 0)
                        for half in range(2):
                            sl = slice(half * 512, (half + 1) * 512)
                            nc.tensor.matmul(q_ps[half], xT[:, c, :],
                                             w_sb["wq"][:, c, sl],
                                             start=st, stop=False)
                            nc.tensor.matmul(k_ps[half], xT[:, c, :],
                                             w_sb["wk"][:, c, sl],
                                             start=st, stop=(c == NC - 1))
                            nc.tensor.matmul(v_ps[half], xT[:, c, :],
                                             w_sb["wv"][:, c, sl],
                                             start=st, stop=(c == NC - 1))
                    # q aug: += s*(-c) + d  (K=2)
                    for half in range(2):
                        sl = slice(half * 512, (half + 1) * 512)
                        nc.tensor.matmul(q_ps[half], augT, cdq_sb[:, sl],
                                         start=False, stop=True)

                    # ---- elu(k) (+bk), elu(q) with rstd folded in ----
                    raw_k = sa.tile([P, NH, HD], F32, tag="rawk", bufs=2,
                                    name=f"rawk{t}")
                    raw_q = sa.tile([P, NH, HD], F32, tag="rawq", bufs=2,
                                    name=f"rawq{t}")
                    v_sb = sa.tile([P, NH, HD], F32R, tag="vsb", bufs=2,
                                   name=f"v_sb{t}")
                    for half in range(2):
                        sl = slice(half * 512, (half + 1) * 512)
                        hh = slice(half * HNH, (half + 1) * HNH)
                        # k path: m = max(kps,0)-1 ; e = exp(min(kps,0))
                        if bk_nz:
                            xb = sa.tile([P, 512], F32, tag="kxb", bufs=2,
                                         name=f"kxb{t}_{half}")
                            nc.vector.tensor_tensor(xb, k_ps[half],
                                                    brep["bk"][:, sl], OP.add)
                            ksrc = xb[:]
                        else:
                            ksrc = k_ps[half][:]
                        m = sa.tile([P, 512], F32, tag="km", bufs=2,
                                    name=f"km{t}_{half}")
                        nc.vector.tensor_scalar(m, ksrc, 0.0, 1.0,
                                                OP.max, OP.subtract)
                        r = sa.tile([P, 512], F32, tag="kr", bufs=2,
                                    name=f"kr{t}_{half}")
                        nc.scalar.activation(r, ksrc, AF.Relu, scale=-1.0)
                        e = sa.tile([P, 512], F32, tag="ke", bufs=2,
                                    name=f"ke{t}_{half}")
                        nc.scalar.activation(e, r, AF.Exp, scale=-1.0)
                        nc.gpsimd.tensor_tensor(
                            raw_k[:, hh, :].rearrange("p h d -> p (h d)"),
                            m, e, OP.add)
                        # q path: m = max(rstd*qps,0) ; e = exp(min(rstd*qps,0))
                        mq = sa.tile([P, 512], F32, tag="qm", bufs=2,
                                     name=f"qm{t}_{half}")
                        nc.vector.tensor_scalar(mq, q_ps[half], rstd, 0.0,
                                                OP.mult, OP.max)
                        rq = sa.tile([P, 512], F32, tag="qr", bufs=2,
                                     name=f"qr{t}_{half}")
                        nc.scalar.activation(rq, q_ps[half], AF.Relu,
                                             scale=nrstd[:, 0:1])
                        eq_ = sa.tile([P, 512], F32, tag="qe", bufs=2,
                                      name=f"qe{t}_{half}")
                        nc.scalar.activation(eq_, rq, AF.Exp, scale=-1.0)
                        rawh = raw_q[:, hh, :].rearrange("p h d -> p (h d)")
                        nc.gpsimd.tensor_tensor(rawh, mq, eq_, OP.add)
                        nc.gpsimd.tensor_scalar(rawh, rawh, 1.0, None,
                                                OP.subtract)
                        # v path
                        vdst = v_sb[:, hh, :].rearrange("p h d -> p (h d)")
                        if bv_nz:
                            nc.vector.tensor_tensor(
                                vdst, v_ps[half], brep["bv"][:, sl], OP.add)
                        else:
                            nc.scalar.copy(vdst, v_ps[half])

                    # ---- per-head l2 norms; scale ----
                    ek = sa.tile([P, NH, HD], F32R, tag="ek", bufs=2,
                                 name=f"ek{t}")
                    eqs = sa.tile([P, NH, HD], F32R, tag="eq", bufs=2,
                                  name=f"eqs{t}")
                    for nm, raw, dst in (("k", raw_k, ek), ("q", raw_q, eqs)):
                        sq = sa.tile([P, NH, HD], F32, tag="sq", bufs=2,
                                     name=f"sq{nm}{t}")
                        nc.scalar.activation(
                            sq[:].rearrange("p h d -> p (h d)"),
                            raw[:].rearrange("p h d -> p (h d)"), AF.Square)
                        ss = sa.tile([P, NH], F32, tag=f"ss{nm}", bufs=4,
                                     name=f"ss{nm}{t}")
                        nc.vector.tensor_reduce(ss, sq, mybir.AxisListType.X,
                                                OP.add)
                        rn = _rsqrt(nc, sa, consts, ss[:], NH, f"{nm}rn{t}")
                        nc.vector.tensor_tensor(
                            dst, raw, rn[:, :, None].to_broadcast((P, NH, HD)),
                            OP.mult)
                    if debug and t == 0:
                        nc.sync.dma_start(
                            dbg["ek0"].ap(),
                            ek[:].rearrange("p h d -> p (h d)").bitcast(F32))
                        nc.sync.dma_start(
                            dbg["v0"].ap(),
                            v_sb[:].rearrange("p h d -> p (h d)").bitcast(F32))

                    # ---- kv state accumulation (2 banks, partitions 0:64)
                    for h in range(NH):
                        nc.tensor.matmul(
                            kv_ps[:, h * HD:(h + 1) * HD],
                            ek[:, h, :], v_sb[:, h, :],
                            start=(t == 0 and h % 8 == 0), stop=(t == NT - 1),
                            skip_group_check=True)

                    # ---- transpose eq -> bf16 spill ----
                    eqf = eqs[:].rearrange("p h d -> p (h d)")
                    if debug and t == 0:
                        nc.sync.dma_start(dbg["eq0"].ap(), eqf.bitcast(F32))
                    eqt_sb = sa.tile([P, NC * P], BF16, tag="eqt", bufs=2,
                                     name=f"eqt{t}")
                    for half in range(2):
                        tp = pa.tile([P, 4 * P], F32, tag="A", bufs=2,
                                     name=f"tpB_{t}_{half}")
                        for b_ in range(4):
                            blk = half * 4 + b_
                            nc.tensor.transpose(
                                tp[:, b_ * P:(b_ + 1) * P].bitcast(F32R),
                                eqf[:, blk * P:(blk + 1) * P], ident_r)
                        nc.vector.tensor_copy(
                            eqt_sb[:, half * 4 * P:(half + 1) * 4 * P], tp)
                    nc.sync.dma_start(eqt_d.ap()[t], eqt_sb)

                # kv state -> SBUF bf16, packed 2 heads per 128 partitions
                kvv = kv_ps[:].rearrange("p (a r d) -> p a r d", r=2, d=HD)
                kvb = kv_sb[:].rearrange("p (a d) -> p a d", d=HD)
                nc.vector.tensor_copy(kvb[0:HD], kvv[:, :, 0, :])
                nc.vector.tensor_copy(kvb[HD:P], kvv[:, :, 1, :])
                if debug:
                    kvstage = sa.tile([HD, NH * HD], F32, name="kvstage")
                    nc.vector.tensor_copy(kvstage, kv_ps)
                    nc.sync.dma_start(dbg["kv"].ap(), kvstage)

            # ---------------- pass B: ctx -> out ---------------------------
            with tc.tile_pool(name="sbufB", bufs=1) as sb, \
                 tc.tile_pool(name="psumB", bufs=1, space="PSUM") as pb:
                wo_sb = wpool.tile([P, NC, HID], F32R, tag="w_wq",
                                   name="wo_sb")
                nc.sync.dma_start(
                    wo_sb,
                    wot_d.ap().rearrange("(c p) j -> p c j", p=P).bitcast(F32R))
                for ch in range(NCHUNKS):
                    eqt_ld = sb.tile([P, NC, CHUNK, P], BF16, tag="eqld",
                                     bufs=2, name=f"eqld{ch}")
                    for tl in range(CHUNK):
                        t = ch * CHUNK + tl
                        nc.scalar.dma_start(
                            eqt_ld[:, :, tl, :],
                            eqt_d.ap()[t].rearrange("p (c s) -> p c s", s=P))

                    # ctx^T per head pair (one bank, col/row-tiled 64-K pair)
                    ctxT = sb.tile([P, NC, CHUNK * P], F32R, tag="ctxT",
                                   bufs=1, name=f"ctxT{ch}")
                    for jt in range(NC):
                        cs = slice(jt * HD, (jt + 1) * HD)
                        c_pse = pb.tile([HD, CHUNK * P], F32, tag="ctxe",
                                        bufs=2, name=f"c_pse{ch}_{jt}")
                        c_pso = pb.tile([HD, CHUNK * P], F32, tag="ctxo",
                                        bufs=2, name=f"c_pso{ch}_{jt}")
                        nc.tensor.matmul(c_pse, kv_sb[0:HD, cs],
                                         eqt_ld[0:HD, jt, :, :],
                                         start=True, stop=True)
                        nc.tensor.matmul(c_pso, kv_sb[HD:P, cs],
                                         eqt_ld[HD:P, jt, :, :],
                                         start=True, stop=True)
                        nc.scalar.copy(ctxT[0:HD, jt, :], c_pse)
                        nc.vector.tensor_copy(ctxT[HD:P, jt, :], c_pso)

                    for tl in range(CHUNK):
                        t = ch * CHUNK + tl

                        x_t2 = sb.tile([P, HID], F32, tag="x2", bufs=3,
                                       name=f"x2_{t}")
                        nc.sync.dma_start(
                            x_t2, x_d.ap()[t * P:(t + 1) * P, :])
                        if bo_nz:
                            xb2 = sb.tile([P, HID], F32, tag="xb2", bufs=2,
                                          name=f"xb2_{t}")
                            nc.gpsimd.tensor_tensor(xb2, x_t2, brep["bo"],
                                                    OP.add)
                            res = xb2
                        else:
                            res = x_t2
                        out_sb = sb.tile([P, HID], F32, tag="osb", bufs=2,
                                         name=f"out_{t}")
                        for half in range(2):
                            sl = slice(half * 512, (half + 1) * 512)
                            o_ps = pb.tile([P, 512], F32, tag="oh", bufs=4,
                                           name=f"o_ps{t}_{half}")
                            for c in range(NC):
                                nc.tensor.matmul(
                                    o_ps, ctxT[:, c, tl * P:(tl + 1) * P],
                                    wo_sb[:, c, sl],
                                    start=(c == 0), stop=(c == NC - 1))
                            nc.vector.tensor_tensor(
                                out_sb[:, sl], o_ps, res[:, sl], OP.add)
                        nc.gpsimd.dma_start(
                            out_d.ap()[t * P:(t + 1) * P, :], out_sb)

    nc.compile()
    return nc


_RUNNER = {}


def _get_runner(loop_n=1, flags=(False, False, False)):
    key = (loop_n, flags)
    if key in _RUNNER:
        return _RUNNER[key]

    import jax
    from jax.sharding import Mesh, PartitionSpec
    from jax.experimental.shard_map import shard_map
    from concourse.bass2jax import _bass_exec_p, install_neuronx_cc_hook

    install_neuronx_cc_hook()
    nc = build_nc(loop_n=loop_n, bk_nz=flags[0], bv_nz=flags[1],
                  bo_nz=flags[2])

    in_names = []
    out_names = []
    out_avals = []
    for alloc in nc.m.functions[0].allocations:
        if not isinstance(alloc, mybir.MemoryLocationSet):
            continue
        name = alloc.memorylocations[0].name
        if alloc.kind == "ExternalInput":
            in_names.append(name)
        elif alloc.kind == "ExternalOutput":
            out_names.append(name)
            out_avals.append(
                jax.core.ShapedArray(tuple(alloc.tensor_shape),
                                     mybir.dt.np(alloc.dtype)))
    n_params = len(in_names)
    all_in_names = in_names + out_names

    def _body(*args):
        outs = _bass_exec_p.bind(
            *args,
            out_avals=tuple(out_avals),
            in_names=tuple(all_in_names),
            out_names=tuple(out_names),
            lowering_input_output_aliases=(),
            sim_require_finite=True,
            sim_require_nnan=True,
            nc=nc,
        )
        return tuple(outs)

    devices = jax.devices()[:B]
    mesh = Mesh(np.asarray(devices), ("core",))
    n_outs = len(out_names)
    fn = jax.jit(
        shard_map(
            _body, mesh=mesh,
            in_specs=(PartitionSpec("core"),) * (n_params + n_outs),
            out_specs=(PartitionSpec("core"),) * n_outs,
            check_rep=False,
        ),
        keep_unused=True,
    )
    _RUNNER[key] = (fn, in_names, out_names, out_avals)
    return _RUNNER[key]


def prep_inputs(input_tensor, attention_mask, ln_gamma, ln_beta,
                Wq, bq, Wk, bk, Wv, bv, Wo, bo):
    """Host-side static prep: transpose weights, fold LN into q projection."""
    f = np.float32
    x = np.ascontiguousarray(np.asarray(input_tensor, f))
    g = np.asarray(ln_gamma, f)
    be = np.asarray(ln_beta, f)
    Wq = np.asarray(Wq, f); Wk = np.asarray(Wk, f)
    Wv = np.asarray(Wv, f); Wo = np.asarray(Wo, f)
    wg = np.ascontiguousarray((Wq * g[None, :]).T)          # [i, j]
    c = wg.sum(axis=0)                                      # [j]
    d = (np.asarray(bq, f) + be @ Wq.T).astype(f)           # [j]
    cdq = np.stack([-c, d], axis=0).astype(f)               # [2, j]
    wkt = np.ascontiguousarray(Wk.T)
    wvt = np.ascontiguousarray(Wv.T)
    wot = np.ascontiguousarray(Wo.T * np.float32(1.0 / np.sqrt(HD)))
    per_core = {
        "wqt": wg, "wkt": wkt, "wvt": wvt, "wot": wot,
        "cdq": cdq,
        "bk": np.asarray(bk, f).reshape(1, HID),
        "bv": np.asarray(bv, f).reshape(1, HID),
        "bo": np.asarray(bo, f).reshape(1, HID),
    }
    return x, per_core


def kernel(**inputs) -> np.ndarray:
    x, per_core = prep_inputs(**inputs)
    flags = (bool(np.any(per_core["bk"])), bool(np.any(per_core["bv"])),
             bool(np.any(per_core["bo"])))
    fn, in_names, out_names, out_avals = _get_runner(1, flags)

    concat_in = []
    for name in in_names:
        if name == "x":
            concat_in.append(x.reshape(B * S, HID))
        else:
            concat_in.append(np.concatenate([per_core[name]] * B, axis=0))
    concat_zeros = [
        np.zeros((B * av.shape[0], *av.shape[1:]), av.dtype) for av in out_avals
    ]
    out_arrs = fn(*concat_in, *concat_zeros)
    out = np.asarray(out_arrs[out_names.index("out")])
    return out.reshape(B, S, HID)


# revision 15
# speedup vs baseline: 1.2921x; 1.2921x over previous
"""Trainium2 Bass kernel for nn_MultiHeadAttention_84791244358011.

Linear (ELU feature-map) attention:
    x_norm = LayerNorm(x)                      # eps=1e-12
    q = x_norm @ Wq.T + bq ; k,v = x @ W.T + b # per-head [S, 64]
    eq/ek = l2norm(elu(q/k)) per token over head_dim
    kv = ek^T @ v per head [64, 64]; ctx = eq @ kv / 8
    out = ctx @ Wo.T + bo + x

Sharding: data-parallel over batch B=8 — one batch element per NeuronCore,
no collectives.

v3 design: LayerNorm folded into the q projection algebraically:
    q = rstd * (x @ Wg^T  - mu*c + std*d)
with Wg[i,j] = gamma[i]*Wq[j,i], c = colsums(Wg), d = bq + beta@Wq.T,
std = (var+eps)*rstd. q/k/v all project from the same transposed-x
stationary tiles in ONE pass (no z tensor / zT spill). The [mu;std] aug
rows are built by two tiny SBUF->SBUF DMAs (no PE transpose, no ACT Sqrt
— keeps the ACT table set fixed on exp_and_others). The per-head l2
rsqrt for q and k is one batched Newton call on [128,32].

PE software pipelining (PE executes its stream in order): per tile t the
PE stream is [x-transposes(t), 48 proj MMs(t), 2 aug MMs(t), kv MMs(t-1),
eq-transposes(t-1)] so the elu/l2norm latency of tile t hides under tile
t+1's projection matmuls. Pass B likewise defers each chunk's output
projection behind the next chunk's ctx matmuls.

PSUM: tag A = 4 banks (x-transposes + q/v halves), tag B = 2 banks
(k halves + deferred eq-transposes), kv state = 2 banks.
"""

import numpy as np

import concourse.bass as bass
import concourse.mybir as mybir
import concourse.tile as tile
from concourse import bacc
from concourse.masks import make_identity

B, S, HID = 8, 4096, 1024
NH, HD = 16, 64
P = 128
NT = S // P            # 32 token tiles
NC = HID // P          # 8 feature chunks
HNH = NH // 2          # heads per half
CHUNK = 4              # token tiles per ctx chunk (512 tokens)
NCHUNKS = NT // CHUNK
LN_EPS = 1e-12

F32 = mybir.dt.float32
F32R = mybir.dt.float32r
BF16 = mybir.dt.bfloat16
I32 = mybir.dt.int32
AF = mybir.ActivationFunctionType
OP = mybir.AluOpType


def _rsqrt(nc, pool, consts, src, n, name, iters=2):
    """1/sqrt(src[:, :n]) on DVE: quake-III seed + Newton iterations."""
    magic_b, one_b = consts
    e = nc.vector
    shi = pool.tile([P, n], I32, tag=f"rq_sh{n}", bufs=4, name=f"{name}_shi")
    e.tensor_tensor(shi, src.bitcast(I32), one_b[:, 0:1].to_broadcast((P, n)),
                    OP.arith_shift_right)
    y0i = pool.tile([P, n], I32, tag=f"rq_y0{n}", bufs=4, name=f"{name}_y0i")
    e.tensor_tensor(y0i, magic_b[:, 0:1].to_broadcast((P, n)), shi, OP.subtract)
    h = pool.tile([P, n], F32, tag=f"rq_h{n}", bufs=4, name=f"{name}_h")
    e.tensor_scalar(h, src, -0.5, None, OP.mult)
    cur = y0i[:].bitcast(F32)
    for it in range(iters):
        t = pool.tile([P, n], F32, tag=f"rq_t{n}_{it}", bufs=4,
                      name=f"{name}_t{it}")
        e.tensor_tensor(t, cur, cur, OP.mult)
        e.tensor_tensor(t, t, h, OP.mult)
        e.tensor_scalar(t, t, 1.5, None, OP.add)
        y = pool.tile([P, n], F32, tag=f"rq_y{n}_{it}", bufs=4,
                      name=f"{name}_y{it}")
        e.tensor_tensor(y, cur, t, OP.mult)
        cur = y
    return cur


def build_nc(loop_n=1, bk_nz=False, bv_nz=False, bo_nz=False, debug=False):
    nc = bacc.Bacc("TRN2", target_bir_lowering=False, enable_partition_id=False)
    dbg = {}
    if debug:
        dbg["ek0"] = nc.dram_tensor("dbg_ek0", [P, HID], F32, kind="ExternalOutput")
        dbg["v0"] = nc.dram_tensor("dbg_v0", [P, HID], F32, kind="ExternalOutput")
        dbg["kv"] = nc.dram_tensor("dbg_kv", [HD, NH * HD], F32,
                                   kind="ExternalOutput")
        dbg["eq0"] = nc.dram_tensor("dbg_eq0", [P, HID], F32, kind="ExternalOutput")

    x_d = nc.dram_tensor("x", [S, HID], F32, kind="ExternalInput")
    wqt_d = nc.dram_tensor("wqt", [HID, HID], F32, kind="ExternalInput")
    wkt_d = nc.dram_tensor("wkt", [HID, HID], F32, kind="ExternalInput")
    wvt_d = nc.dram_tensor("wvt", [HID, HID], F32, kind="ExternalInput")
    wot_d = nc.dram_tensor("wot", [HID, HID], F32, kind="ExternalInput")
    cdq_d = nc.dram_tensor("cdq", [2, HID], F32, kind="ExternalInput")
    bk_d = nc.dram_tensor("bk", [1, HID], F32, kind="ExternalInput")
    bv_d = nc.dram_tensor("bv", [1, HID], F32, kind="ExternalInput")
    bo_d = nc.dram_tensor("bo", [1, HID], F32, kind="ExternalInput")
    out_d = nc.dram_tensor("out", [S, HID], F32, kind="ExternalOutput")
    eqt_d = nc.dram_tensor("eqt_spill", [NT, P, NC * P], BF16)

    import contextlib

    with tile.TileContext(nc) as tc, contextlib.ExitStack() as ctx:
        persist = ctx.enter_context(tc.tile_pool(name="persist", bufs=1))
        wpool = ctx.enter_context(tc.tile_pool(name="weights", bufs=1))

        ident = persist.tile([P, P], F32)
        make_identity(nc, ident)
        ident_r = persist.tile([P, P], F32R)
        nc.scalar.activation(ident_r, ident, AF.Copy)
        magic_t = persist.tile([P, 1], I32)
        nc.gpsimd.memset(magic_t, 0x5F3759DF)
        one_t = persist.tile([P, 1], I32)
        nc.gpsimd.memset(one_t, 1)
        consts = (magic_t, one_t)
        kv_sb = persist.tile([P, NC * HD], BF16)   # 2 heads per 128 partitions
        cdq_sb = persist.tile([2, HID], F32R, name="cdq_sb")
        nc.sync.dma_start(cdq_sb, cdq_d.ap().bitcast(F32R))
        brep = {}
        reps = []
        if bk_nz:
            reps.append(("bk", bk_d))
        if bv_nz:
            reps.append(("bv", bv_d))
        if bo_nz:
            reps.append(("bo", bo_d))
        for nm, d in reps:
            t_ = persist.tile([P, HID], F32, name=f"{nm}_rep")
            h = d.ap()
            nc.gpsimd.dma_start(
                t_, bass.AP(tensor=h.tensor, offset=h.offset,
                            ap=[[0, P], [1, HID]]))
            brep[nm] = t_

        _loop = tc.For_i(0, loop_n, 1) if loop_n > 1 else contextlib.nullcontext(0)
        with _loop:
            # q/k/v weights resident (f32r, contraction dim on partitions)
            w_sb = {}
            w_src = {}
            for nm, d, eng in (("wk", wkt_d, nc.sync), ("wq", wqt_d, nc.gpsimd),
                               ("wv", wvt_d, nc.scalar)):
                t_ = wpool.tile([P, NC, HID], F32R, tag=f"w_{nm}", name=f"{nm}_sb")
                w_sb[nm] = t_
                w_src[nm] = (d, eng)

            def load_weights():
                # chunked per c-block so the first projection chunks arrive
                # early; queues: wk->SP, wq->SWDGE, wv->ACT
                for c in range(NC):
                    for nm, (d, eng) in w_src.items():
                        eng.dma_start(
                            w_sb[nm][:, c, :],
                            d.ap().rearrange("(c p) j -> p c j", p=P)[
                                :, c, :].bitcast(F32R))

            # ---------------- pass A: q/k/v + kv state + eqT spill ---------
            with tc.tile_pool(name="sbufA", bufs=1) as sa, \
                 tc.tile_pool(name="psumA", bufs=1, space="PSUM") as pa:
                kv_ps = pa.tile([HD, NH * HD], F32, tag="kv", name="kv_ps")

                deferred = [None]

                def flush_deferred():
                    d = deferred[0]
                    if d is None:
                        return
                    td, ek, v_sb, eqs = d
                    deferred[0] = None
                    # kv state accumulation (partitions 0:64, 2 banks)
                    for h in range(NH):
                        nc.tensor.matmul(
                            kv_ps[:, h * HD:(h + 1) * HD],
                            ek[:, h, :], v_sb[:, h, :],
                            start=(td == 0 and h % 8 == 0),
                            stop=(td == NT - 1),
                            skip_group_check=True)
                    # transpose eq -> bf16 spill (tag B banks)
                    eqf = eqs[:].rearrange("p h d -> p (h d)")
                    eqt_sb = sa.tile([P, NC * P], BF16, tag="eqt", bufs=2,
                                     name=f"eqt{td}")
                    for half in range(2):
                        tp = pa.tile([P, 4 * P], F32, tag="B", bufs=2,
                                     name=f"tpB_{td}_{half}")
                        for b_ in range(4):
                            blk = half * 4 + b_
                            nc.tensor.transpose(
                                tp[:, b_ * P:(b_ + 1) * P].bitcast(F32R),
                                eqf[:, blk * P:(blk + 1) * P], ident_r)
                        nc.scalar.copy(
                            eqt_sb[:, half * 4 * P:(half + 1) * 4 * P], tp)
                    nc.sync.dma_start(eqt_d.ap()[td], eqt_sb)

                def load_x(t):
                    x_t = sa.tile([P, HID], F32R, tag="x", bufs=3,
                                  name=f"x_{t}")
                    nc.scalar.dma_start(
                        x_t, x_d.ap()[t * P:(t + 1) * P, :].bitcast(F32R))
                    return x_t

                def stats_chain(t, x_t):
                    """DVE: LN stats -> rstd, nrstd, [mu; std] aug rows."""
                    stats = sa.tile([P, 2, 6], F32, tag="st", bufs=2,
                                    name=f"st_{t}")
                    xg = x_t[:].bitcast(F32).rearrange("p (g d) -> p g d",
                                                       g=2)
                    for g in range(2):
                        nc.vector.bn_stats(stats[:, g, :], xg[:, g, :])
                    mv = sa.tile([P, 2], F32, tag="mv", bufs=4, name=f"mv_{t}")
                    nc.vector.bn_aggr(mv, stats)
                    vpe = sa.tile([P, 1], F32, tag="sd", bufs=4, name=f"sd_{t}")
                    nc.vector.tensor_scalar(vpe, mv[:, 1:2], LN_EPS, None,
                                            OP.add)
                    rstd = _rsqrt(nc, sa, consts, vpe[:], 1, f"rs_{t}")
                    nrstd = sa.tile([P, 1], F32, tag="nrs", bufs=4,
                                    name=f"nrs_{t}")
                    nc.vector.tensor_scalar(nrstd, rstd, -1.0, None, OP.mult)
                    # s2 = [mu, std]: psum gets -mu*c + std*d; the rstd scale
                    # later turns it into -mu*rstd*c + d.  std = (var+eps)*rstd
                    s2 = sa.tile([P, 2], F32R, tag="s2", bufs=4,
                                 name=f"s2_{t}")
                    nc.vector.tensor_copy(s2[:, 0:1], mv[:, 0:1])
                    nc.vector.tensor_tensor(s2[:, 1:2], vpe, rstd, OP.mult)
                    # aug rows via two tiny SBUF->SBUF DMA transposes
                    augT = sa.tile([2, P], F32R, tag="augT", bufs=4,
                                   name=f"augT_{t}")
                    nc.gpsimd.dma_start(augT[0:1, :], s2[:, 0:1])
                    nc.gpsimd.dma_start(augT[1:2, :], s2[:, 1:2])
                    return rstd, nrstd, augT

                def transpose_x(t, x_t):
                    """PE transposes + ACT copies -> xT (tag A banks)."""
                    xT = sa.tile([P, NC, P], F32R, tag="xT", bufs=2,
                                 name=f"xT_{t}")
                    for half in range(2):
                        tp = pa.tile([P, 4 * P], F32, tag="A", bufs=4,
                                     name=f"tp_{t}_{half}")
                        for b_ in range(4):
                            blk = half * 4 + b_
                            nc.tensor.transpose(
                                tp[:, b_ * P:(b_ + 1) * P].bitcast(F32R),
                                x_t[:, blk * P:(blk + 1) * P], ident_r)
                        nc.scalar.copy(
                            xT[:].rearrange("p c s -> p (c s)")[
                                :, half * 4 * P:(half + 1) * 4 * P], tp)
                    return xT

                x_cur = load_x(0)
                x_nxt = load_x(1)
                load_weights()
                pre = stats_chain(0, x_cur)
                xT_cur = transpose_x(0, x_cur)
                for t in range(NT):
                    rstd, nrstd, augT = pre
                    xT = xT_cur

                    # q/k/v projections over the shared xT stationary.
                    q_ps = [pa.tile([P, 512], F32, tag="A", bufs=4,
                                    name=f"q_ps{t}_{h}") for h in range(2)]
                    v_ps = [pa.tile([P, 512], F32, tag="A", bufs=4,
                                    name=f"v_ps{t}_{h}") for h in range(2)]
                    k_ps = [pa.tile([P, 512], F32, tag="B", bufs=2,
                                    name=f"k_ps{t}_{h}") for h in range(2)]
                    # k+q first so their psum banks release early; v after
                    for c in range(NC):
                        st = (c == 0)
                        for half in range(2):
                            sl = slice(half * 512, (half + 1) * 512)
                            nc.tensor.matmul(k_ps[half], xT[:, c, :],
                                             w_sb["wk"][:, c, sl],
                                             start=st, stop=(c == NC - 1))
                            nc.tensor.matmul(q_ps[half], xT[:, c, :],
                                             w_sb["wq"][:, c, sl],
                                             start=st, stop=False)
                    # q aug: += mu*(-c) + std*d  (K=2)
                    for half in range(2):
                        sl = slice(half * 512, (half + 1) * 512)
                        nc.tensor.matmul(q_ps[half], augT, cdq_sb[:, sl],
                                         start=False, stop=True)
                    for c in range(NC):
                        for half in range(2):
                            sl = slice(half * 512, (half + 1) * 512)
                            nc.tensor.matmul(v_ps[half], xT[:, c, :],
                                             w_sb["wv"][:, c, sl],
                                             start=(c == 0),
                                             stop=(c == NC - 1))

                    # lookahead: next tile's stats run on DVE while this
                    # tile's projections stream on PE
                    if t + 1 < NT:
                        pre = stats_chain(t + 1, x_nxt)

                    # ---- elu(k) (+bk), elu(q) with rstd folded in ----
                    # PSUM-reading ops are emitted FIRST on each engine so
                    # the q/k/v psum banks release as early as possible.
                    raw_k = sa.tile([P, NH, HD], F32, tag="rawk", bufs=2,
                                    name=f"rawk{t}")
                    raw_q = sa.tile([P, NH, HD], F32, tag="rawq", bufs=2,
                                    name=f"rawq{t}")
                    v_sb = sa.tile([P, NH, HD], F32R, tag="vsb", bufs=2,
                                   name=f"v_sb{t}")
                    m = [None, None]
                    r = [None, None]
                    mq = [None, None]
                    rq = [None, None]
                    ksrc = [None, None]
                    for half in range(2):
                        sl = slice(half * 512, (half + 1) * 512)
                        if bk_nz:
                            xb = sa.tile([P, 512], F32, tag="kxb", bufs=2,
                                         name=f"kxb{t}_{half}")
                            nc.vector.tensor_tensor(xb, k_ps[half],
                                                    brep["bk"][:, sl], OP.add)
                            ksrc[half] = xb[:]
                        else:
                            ksrc[half] = k_ps[half][:]
                        m[half] = sa.tile([P, 512], F32, tag="km", bufs=2,
                                          name=f"km{t}_{half}")
                        nc.vector.tensor_scalar(m[half], ksrc[half], 0.0, 1.0,
                                                OP.max, OP.subtract)
                        r[half] = sa.tile([P, 512], F32, tag="kr", bufs=2,
                                          name=f"kr{t}_{half}")
                        nc.scalar.activation(r[half], ksrc[half], AF.Relu,
                                             scale=-1.0)
                    for half in range(2):
                        mq[half] = sa.tile([P, 512], F32, tag="qm", bufs=2,
                                           name=f"qm{t}_{half}")
                        nc.vector.tensor_scalar(mq[half], q_ps[half], rstd,
                                                0.0, OP.mult, OP.max)
                        rq[half] = sa.tile([P, 512], F32, tag="qr", bufs=2,
                                           name=f"qr{t}_{half}")
                        nc.scalar.activation(rq[half], q_ps[half], AF.Relu,
                                             scale=nrstd[:, 0:1])
                    # PE: deferred kv + eq-transpose of the previous tile,
                    # then next tile's x transposes (banks just released)
                    flush_deferred()
                    if t + 1 < NT:
                        xT_cur = transpose_x(t + 1, x_nxt)
                        x_cur = x_nxt
                        if t + 2 < NT:
                            x_nxt = load_x(t + 2)

                    for half in range(2):
                        sl = slice(half * 512, (half + 1) * 512)
                        hh = slice(half * HNH, (half + 1) * HNH)
                        vdst = v_sb[:, hh, :].rearrange("p h d -> p (h d)")
                        if bv_nz:
                            nc.vector.tensor_tensor(
                                vdst, v_ps[half], brep["bv"][:, sl], OP.add)
                        else:
                            nc.scalar.copy(vdst, v_ps[half])

                    # SBUF-only tail: exp, combine, squares
                    for half in range(2):
                        hh = slice(half * HNH, (half + 1) * HNH)
                        e = sa.tile([P, 512], F32, tag="ke", bufs=2,
                                    name=f"ke{t}_{half}")
                        nc.scalar.activation(e, r[half], AF.Exp, scale=-1.0)
                        nc.gpsimd.tensor_tensor(
                            raw_k[:, hh, :].rearrange("p h d -> p (h d)"),
                            m[half], e, OP.add)
                        eq_ = sa.tile([P, 512], F32, tag="qe", bufs=2,
                                      name=f"qe{t}_{half}")
                        nc.scalar.activation(eq_, rq[half], AF.Exp, scale=-1.0)
                        rawh = raw_q[:, hh, :].rearrange("p h d -> p (h d)")
                        nc.gpsimd.tensor_tensor(rawh, mq[half], eq_, OP.add)
                        nc.gpsimd.tensor_scalar(rawh, rawh, 1.0, None,
                                                OP.subtract)

                    # ---- per-head l2 norms (one batched rsqrt) + scale ----
                    sq = sa.tile([P, NH, HD], F32, tag="sq", bufs=2,
                                 name=f"sq{t}")
                    ss = sa.tile([P, 2, NH], F32, tag="ss", bufs=4,
                                 name=f"ss{t}")
                    nc.gpsimd.tensor_tensor(
                        sq[:].rearrange("p h d -> p (h d)"),
                        raw_k[:].rearrange("p h d -> p (h d)"),
                        raw_k[:].rearrange("p h d -> p (h d)"), OP.mult)
                    nc.vector.tensor_reduce(ss[:, 0, :], sq,
                                            mybir.AxisListType.X, OP.add)
                    sq2 = sa.tile([P, NH, HD], F32, tag="sq", bufs=2,
                                  name=f"sq2{t}")
                    nc.gpsimd.tensor_tensor(
                        sq2[:].rearrange("p h d -> p (h d)"),
                        raw_q[:].rearrange("p h d -> p (h d)"),
                        raw_q[:].rearrange("p h d -> p (h d)"), OP.mult)
                    nc.vector.tensor_reduce(ss[:, 1, :], sq2,
                                            mybir.AxisListType.X, OP.add)
                    rn = _rsqrt(nc, sa, consts,
                                ss[:].rearrange("p a h -> p (a h)"), 2 * NH,
                                f"rn{t}")
                    ek = sa.tile([P, NH, HD], F32R, tag="ek", bufs=2,
                                 name=f"ek{t}")
                    eqs = sa.tile([P, NH, HD], F32R, tag="eq", bufs=2,
                                  name=f"eqs{t}")
                    nc.vector.tensor_tensor(
                        ek, raw_k,
                        rn[:, 0:NH][:, :, None].to_broadcast((P, NH, HD)),
                        OP.mult)
                    nc.vector.tensor_tensor(
                        eqs, raw_q,
                        rn[:, NH:2 * NH][:, :, None].to_broadcast((P, NH, HD)),
                        OP.mult)
                    if debug and t == 0:
                        nc.sync.dma_start(
                            dbg["ek0"].ap(),
                            ek[:].rearrange("p h d -> p (h d)").bitcast(F32))
                        nc.sync.dma_start(
                            dbg["v0"].ap(),
                            v_sb[:].rearrange("p h d -> p (h d)").bitcast(F32))
                        nc.sync.dma_start(
                            dbg["eq0"].ap(),
                            eqs[:].rearrange("p h d -> p (h d)").bitcast(F32))

                    deferred[0] = (t, ek, v_sb, eqs)
                flush_deferred()

                # kv state -> SBUF bf16, packed 2 heads per 128 partitions
                kvv = kv_ps[:].rearrange("p (a r d) -> p a r d", r=2, d=HD)
                kvb = kv_sb[:].rearrange("p (a d) -> p a d", d=HD)
                nc.vector.tensor_copy(kvb[0:HD], kvv[:, :, 0, :])
                nc.vector.tensor_copy(kvb[HD:P], kvv[:, :, 1, :])
                if debug:
                    kvstage = sa.tile([HD, NH * HD], F32, name="kvstage")
                    nc.vector.tensor_copy(kvstage, kv_ps)
                    nc.sync.dma_start(dbg["kv"].ap(), kvstage)

            # ---------------- pass B: ctx -> out ---------------------------
            with tc.tile_pool(name="sbufB", bufs=1) as sb, \
                 tc.tile_pool(name="psumB", bufs=1, space="PSUM") as pb:
                wo_sb = wpool.tile([P, NC, HID], F32R, tag="w_wq",
                                   name="wo_sb")
                nc.sync.dma_start(
                    wo_sb,
                    wot_d.ap().rearrange("(c p) j -> p c j", p=P).bitcast(F32R))

                deferred_o = [None]

                def flush_o():
                    d = deferred_o[0]
                    if d is None:
                        return
                    chd, ctxT = d
                    deferred_o[0] = None
                    for tl in range(CHUNK):
                        t = chd * CHUNK + tl
                        x_t2 = sb.tile([P, HID], F32, tag="x2", bufs=3,
                                       name=f"x2_{t}")
                        nc.sync.dma_start(
                            x_t2, x_d.ap()[t * P:(t + 1) * P, :])
                        if bo_nz:
                            xb2 = sb.tile([P, HID], F32, tag="xb2", bufs=2,
                                          name=f"xb2_{t}")
                            nc.gpsimd.tensor_tensor(xb2, x_t2, brep["bo"],
                                                    OP.add)
                            res = xb2
                        else:
                            res = x_t2
                        out_sb = sb.tile([P, HID], F32, tag="osb", bufs=2,
                                         name=f"out_{t}")
                        for half in range(2):
                            sl = slice(half * 512, (half + 1) * 512)
                            o_ps = pb.tile([P, 512], F32, tag="oh", bufs=4,
                                           name=f"o_ps{t}_{half}")
                            for c in range(NC):
                                nc.tensor.matmul(
                                    o_ps, ctxT[:, c, tl * P:(tl + 1) * P],
                                    wo_sb[:, c, sl],
                                    start=(c == 0), stop=(c == NC - 1))
                            nc.vector.tensor_tensor(
                                out_sb[:, sl], o_ps, res[:, sl], OP.add)
                        nc.gpsimd.dma_start(
                            out_d.ap()[t * P:(t + 1) * P, :], out_sb)

                for ch in range(NCHUNKS):
                    eqt_ld = sb.tile([P, NC, CHUNK, P], BF16, tag="eqld",
                                     bufs=2, name=f"eqld{ch}")
                    for tl in range(CHUNK):
                        t = ch * CHUNK + tl
                        nc.sync.dma_start(
                            eqt_ld[:, :, tl, :],
                            eqt_d.ap()[t].rearrange("p (c s) -> p c s", s=P))

                    # ctx^T per head pair
                    ctxT = sb.tile([P, NC, CHUNK * P], F32R, tag="ctxT",
                                   bufs=2, name=f"ctxT{ch}")
                    for jt in range(NC):
                        cs = slice(jt * HD, (jt + 1) * HD)
                        c_pse = pb.tile([HD, CHUNK * P], F32, tag="ctxe",
                                        bufs=2, name=f"c_pse{ch}_{jt}")
                        c_pso = pb.tile([HD, CHUNK * P], F32, tag="ctxo",
                                        bufs=2, name=f"c_pso{ch}_{jt}")
                        nc.tensor.matmul(c_pse, kv_sb[0:HD, cs],
                                         eqt_ld[0:HD, jt, :, :],
                                         start=True, stop=True)
                        nc.tensor.matmul(c_pso, kv_sb[HD:P, cs],
                                         eqt_ld[HD:P, jt, :, :],
                                         start=True, stop=True)
                        nc.scalar.copy(ctxT[0:HD, jt, :], c_pse)
                        nc.vector.tensor_copy(ctxT[HD:P, jt, :], c_pso)

                    flush_o()
                    deferred_o[0] = (ch, ctxT)
                flush_o()

    nc.compile()
    return nc


_RUNNER = {}


def _get_runner(loop_n=1, flags=(False, False, False)):
    key = (loop_n, flags)
    if key in _RUNNER:
        return _RUNNER[key]

    import jax
    from jax.sharding import Mesh, PartitionSpec
    from jax.experimental.shard_map import shard_map
    from concourse.bass2jax import _bass_exec_p, install_neuronx_cc_hook

    install_neuronx_cc_hook()
    nc = build_nc(loop_n=loop_n, bk_nz=flags[0], bv_nz=flags[1],
                  bo_nz=flags[2])

    in_names = []
    out_names = []
    out_avals = []
    for alloc in nc.m.functions[0].allocations:
        if not isinstance(alloc, mybir.MemoryLocationSet):
            continue
        name = alloc.memorylocations[0].name
        if alloc.kind == "ExternalInput":
            in_names.append(name)
        elif alloc.kind == "ExternalOutput":
            out_names.append(name)
            out_avals.append(
                jax.core.ShapedArray(tuple(alloc.tensor_shape),
                                     mybir.dt.np(alloc.dtype)))
    n_params = len(in_names)
    all_in_names = in_names + out_names

    def _body(*args):
        outs = _bass_exec_p.bind(
            *args,
            out_avals=tuple(out_avals),
            in_names=tuple(all_in_names),
            out_names=tuple(out_names),
            lowering_input_output_aliases=(),
            sim_require_finite=True,
            sim_require_nnan=True,
            nc=nc,
        )
        return tuple(outs)

    devices = jax.devices()[:B]
    mesh = Mesh(np.asarray(devices), ("core",))
    n_outs = len(out_names)
    fn = jax.jit(
        shard_map(
            _body, mesh=mesh,
            in_specs=(PartitionSpec("core"),) * (n_params + n_outs),
            out_specs=(PartitionSpec("core"),) * n_outs,
            check_rep=False,
        ),
        keep_unused=True,
    )
    _RUNNER[key] = (fn, in_names, out_names, out_avals)
    return _RUNNER[key]


def prep_inputs(input_tensor, attention_mask, ln_gamma, ln_beta,
                Wq, bq, Wk, bk, Wv, bv, Wo, bo):
    """Host-side static prep: transpose weights, fold LN into q projection."""
    f = np.float32
    x = np.ascontiguousarray(np.asarray(input_tensor, f))
    g = np.asarray(ln_gamma, f)
    be = np.asarray(ln_beta, f)
    Wq = np.asarray(Wq, f); Wk = np.asarray(Wk, f)
    Wv = np.asarray(Wv, f); Wo = np.asarray(Wo, f)
    wg = np.ascontiguousarray((Wq * g[None, :]).T)          # [i, j]
    c = wg.sum(axis=0)                                      # [j]
    d = (np.asarray(bq, f) + be @ Wq.T).astype(f)           # [j]
    cdq = np.stack([-c, d], axis=0).astype(f)               # [2, j]
    wkt = np.ascontiguousarray(Wk.T)
    wvt = np.ascontiguousarray(Wv.T)
    wot = np.ascontiguousarray(Wo.T * np.float32(1.0 / np.sqrt(HD)))
    per_core = {
        "wqt": wg, "wkt": wkt, "wvt": wvt, "wot": wot,
        "cdq": cdq,
        "bk": np.asarray(bk, f).reshape(1, HID),
        "bv": np.asarray(bv, f).reshape(1, HID),
        "bo": np.asarray(bo, f).reshape(1, HID),
    }
    return x, per_core


def kernel(**inputs) -> np.ndarray:
    x, per_core = prep_inputs(**inputs)
    flags = (bool(np.any(per_core["bk"])), bool(np.any(per_core["bv"])),
             bool(np.any(per_core["bo"])))
    fn, in_names, out_names, out_avals = _get_runner(1, flags)

    concat_in = []
    for name in in_names:
        if name == "x":
            concat_in.append(x.reshape(B * S, HID))
        else:
            concat_in.append(np.concatenate([per_core[name]] * B, axis=0))
    concat_zeros = [
        np.zeros((B * av.shape[0], *av.shape[1:]), av.dtype) for av in out_avals
    ]
    out_arrs = fn(*concat_in, *concat_zeros)
    out = np.asarray(out_arrs[out_names.index("out")])
    return out.reshape(B, S, HID)


# revision 16
# speedup vs baseline: 5.5040x; 4.2598x over previous
"""Trainium2 Bass kernel for nn_MultiHeadAttention_84791244358011.

Linear (ELU feature-map) attention:
    x_norm = LayerNorm(x)                      # eps=1e-12
    q = x_norm @ Wq.T + bq ; k,v = x @ W.T + b # per-head [S, 64]
    eq/ek = l2norm(elu(q/k)) per token over head_dim
    kv = ek^T @ v per head [64, 64]; ctx = eq @ kv / 8
    out = ctx @ Wo.T + bo + x

Sharding: data-parallel over batch B=8 — one batch element per NeuronCore,
no collectives.

v9 design: LayerNorm folded into the q projection algebraically:
    q = rstd * (x @ Wg^T  - mu*c + std*d)
with Wg[i,j] = gamma[i]*Wq[j,i], c = colsums(Wg), d = bq + beta@Wq.T,
std = (var+eps)*rstd. q/k/v all project from the same transposed-x
stationary tiles in ONE pass (no z tensor / zT spill). The [mu;std] aug
rows are built by two tiny SBUF->SBUF DMAs (no ACT table switch off
exp_and_others). One batched 1-Newton-iteration rsqrt serves both
per-head l2 norms.

All q/k/v PSUM accumulators are bank-PAIRED [128,1024] tiles so every
elementwise consumer is a single full-width instruction (halves the
DVE/ACT instruction count and PSUM bank handoffs vs per-half tiles).

PE software pipelining (PE executes its stream in order): per tile t the
PE stream is [proj k,q,aug,v (t), kv MMs(t-1), eq-transposes(t-1),
x-transposes(t+1)] so the elu/l2norm latency of tile t hides under tile
t+1's projection matmuls. Pass B likewise defers each chunk's output
projection behind the next chunk's ctx matmuls.

PSUM: tag A = 2x[128,1024] (q/v/eq-transpose/x-transpose rotation),
tag B = 1x[128,1024] (k), kv state = 2 banks.
"""

import numpy as np

import concourse.bass as bass
import concourse.mybir as mybir
import concourse.tile as tile
from concourse import bacc
from concourse.masks import make_identity

B, S, HID = 8, 4096, 1024
NH, HD = 16, 64
P = 128
NT = S // P            # 32 token tiles
NC = HID // P          # 8 feature chunks
HNH = NH // 2          # heads per half
CHUNK = 4              # token tiles per ctx chunk (512 tokens)
NCHUNKS = NT // CHUNK
LN_EPS = 1e-12

F32 = mybir.dt.float32
F32R = mybir.dt.float32r
BF16 = mybir.dt.bfloat16
I32 = mybir.dt.int32
AF = mybir.ActivationFunctionType
OP = mybir.AluOpType


def _rsqrt(nc, pool, consts, src, n, name, iters=1):
    """1/sqrt(src[:, :n]) on DVE: quake-III seed + Newton iterations."""
    magic_b, one_b = consts
    e = nc.vector
    shi = pool.tile([P, n], I32, tag=f"rq_sh{n}", bufs=4, name=f"{name}_shi")
    e.tensor_tensor(shi, src.bitcast(I32), one_b[:, 0:1].to_broadcast((P, n)),
                    OP.arith_shift_right)
    y0i = pool.tile([P, n], I32, tag=f"rq_y0{n}", bufs=4, name=f"{name}_y0i")
    e.tensor_tensor(y0i, magic_b[:, 0:1].to_broadcast((P, n)), shi, OP.subtract)
    h = pool.tile([P, n], F32, tag=f"rq_h{n}", bufs=4, name=f"{name}_h")
    e.tensor_scalar(h, src, -0.5, None, OP.mult)
    cur = y0i[:].bitcast(F32)
    for it in range(iters):
        t = pool.tile([P, n], F32, tag=f"rq_t{n}_{it}", bufs=4,
                      name=f"{name}_t{it}")
        e.tensor_tensor(t, cur, cur, OP.mult)
        e.tensor_tensor(t, t, h, OP.mult)
        e.tensor_scalar(t, t, 1.5, None, OP.add)
        y = pool.tile([P, n], F32, tag=f"rq_y{n}_{it}", bufs=4,
                      name=f"{name}_y{it}")
        e.tensor_tensor(y, cur, t, OP.mult)
        cur = y
    return cur


def build_nc(loop_n=1, bk_nz=False, bv_nz=False, bo_nz=False, debug=False):
    nc = bacc.Bacc("TRN2", target_bir_lowering=False, enable_partition_id=False)
    dbg = {}
    if debug:
        dbg["ek0"] = nc.dram_tensor("dbg_ek0", [P, HID], F32, kind="ExternalOutput")
        dbg["v0"] = nc.dram_tensor("dbg_v0", [P, HID], F32, kind="ExternalOutput")
        dbg["kv"] = nc.dram_tensor("dbg_kv", [HD, NH * HD], F32,
                                   kind="ExternalOutput")
        dbg["eq0"] = nc.dram_tensor("dbg_eq0", [P, HID], F32, kind="ExternalOutput")

    x_d = nc.dram_tensor("x", [S, HID], F32, kind="ExternalInput")
    wqt_d = nc.dram_tensor("wqt", [HID, HID], F32, kind="ExternalInput")
    wkt_d = nc.dram_tensor("wkt", [HID, HID], F32, kind="ExternalInput")
    wvt_d = nc.dram_tensor("wvt", [HID, HID], F32, kind="ExternalInput")
    wot_d = nc.dram_tensor("wot", [HID, HID], F32, kind="ExternalInput")
    cdq_d = nc.dram_tensor("cdq", [2, HID], F32, kind="ExternalInput")
    bk_d = nc.dram_tensor("bk", [1, HID], F32, kind="ExternalInput")
    bv_d = nc.dram_tensor("bv", [1, HID], F32, kind="ExternalInput")
    bo_d = nc.dram_tensor("bo", [1, HID], F32, kind="ExternalInput")
    out_d = nc.dram_tensor("out", [S, HID], F32, kind="ExternalOutput")
    eqt_d = nc.dram_tensor("eqt_spill", [NT, P, NC * P], BF16)

    import contextlib

    with tile.TileContext(nc) as tc, contextlib.ExitStack() as ctx:
        persist = ctx.enter_context(tc.tile_pool(name="persist", bufs=1))
        wpool = ctx.enter_context(tc.tile_pool(name="weights", bufs=1))

        ident = persist.tile([P, P], F32)
        make_identity(nc, ident)
        ident_r = persist.tile([P, P], F32R)
        nc.scalar.activation(ident_r, ident, AF.Copy)
        magic_t = persist.tile([P, 1], I32)
        nc.gpsimd.memset(magic_t, 0x5F3759DF)
        one_t = persist.tile([P, 1], I32)
        nc.gpsimd.memset(one_t, 1)
        consts = (magic_t, one_t)
        kv_sb = persist.tile([P, NC * HD], BF16)   # 2 heads per 128 partitions
        cdq_sb = persist.tile([2, HID], F32R, name="cdq_sb")
        nc.sync.dma_start(cdq_sb, cdq_d.ap().bitcast(F32R))
        brep = {}
        reps = []
        if bk_nz:
            reps.append(("bk", bk_d))
        if bv_nz:
            reps.append(("bv", bv_d))
        if bo_nz:
            reps.append(("bo", bo_d))
        for nm, d in reps:
            t_ = persist.tile([P, HID], F32, name=f"{nm}_rep")
            h = d.ap()
            nc.gpsimd.dma_start(
                t_, bass.AP(tensor=h.tensor, offset=h.offset,
                            ap=[[0, P], [1, HID]]))
            brep[nm] = t_

        _loop = tc.For_i(0, loop_n, 1) if loop_n > 1 else contextlib.nullcontext(0)
        with _loop:
            # q/k/v weights resident (f32r, contraction dim on partitions)
            w_sb = {}
            w_src = {}
            for nm, d, eng in (("wk", wkt_d, nc.sync), ("wq", wqt_d, nc.gpsimd),
                               ("wv", wvt_d, nc.scalar)):
                t_ = wpool.tile([P, NC, HID], F32R, tag=f"w_{nm}", name=f"{nm}_sb")
                w_sb[nm] = t_
                w_src[nm] = (d, eng)

            def load_weights():
                # chunked per c-block so the first projection chunks arrive
                # early; queues: wk->SP, wq->SWDGE, wv->ACT
                for c in range(NC):
                    for nm, (d, eng) in w_src.items():
                        eng.dma_start(
                            w_sb[nm][:, c, :],
                            d.ap().rearrange("(c p) j -> p c j", p=P)[
                                :, c, :].bitcast(F32R))

            # ---------------- pass A: q/k/v + kv state + eqT spill ---------
            with tc.tile_pool(name="sbufA", bufs=1) as sa, \
                 tc.tile_pool(name="psumA", bufs=1, space="PSUM") as pa:
                kv_ps = pa.tile([HD, NH * HD], F32, tag="kv", name="kv_ps")

                deferred = [None]

                def flush_deferred():
                    d = deferred[0]
                    if d is None:
                        return
                    td, ek, v_sb, eqs = d
                    deferred[0] = None
                    # kv state accumulation (partitions 0:64, 2 banks)
                    for h in range(NH):
                        nc.tensor.matmul(
                            kv_ps[:, h * HD:(h + 1) * HD],
                            ek[:, h, :], v_sb[:, h, :],
                            start=(td == 0 and h % 8 == 0),
                            stop=(td == NT - 1),
                            skip_group_check=True)
                    # transpose eq -> bf16 spill (one paired-bank tile)
                    eqf = eqs[:].rearrange("p h d -> p (h d)")
                    eqt_sb = sa.tile([P, NC * P], BF16, tag="eqt", bufs=2,
                                     name=f"eqt{td}")
                    tp = pa.tile([P, NC * P], F32, tag="A", bufs=2,
                                 name=f"tpB_{td}")
                    for blk in range(NC):
                        nc.tensor.transpose(
                            tp[:, blk * P:(blk + 1) * P].bitcast(F32R),
                            eqf[:, blk * P:(blk + 1) * P], ident_r)
                    nc.scalar.copy(eqt_sb, tp)
                    nc.sync.dma_start(eqt_d.ap()[td], eqt_sb)

                def load_x(t):
                    x_t = sa.tile([P, HID], F32R, tag="x", bufs=3,
                                  name=f"x_{t}")
                    nc.scalar.dma_start(
                        x_t, x_d.ap()[t * P:(t + 1) * P, :].bitcast(F32R))
                    return x_t

                def stats_chain(t, x_t):
                    """LN stats -> rstd, nrstd, [mu; std] aug rows."""
                    stats = sa.tile([P, 2, 6], F32, tag="st", bufs=2,
                                    name=f"st_{t}")
                    xg = x_t[:].bitcast(F32).rearrange("p (g d) -> p g d",
                                                       g=2)
                    for g in range(2):
                        nc.vector.bn_stats(stats[:, g, :], xg[:, g, :])
                    mv = sa.tile([P, 2], F32, tag="mv", bufs=4, name=f"mv_{t}")
                    nc.vector.bn_aggr(mv, stats)
                    vpe = sa.tile([P, 1], F32, tag="sd", bufs=4, name=f"sd_{t}")
                    nc.vector.tensor_scalar(vpe, mv[:, 1:2], LN_EPS, None,
                                            OP.add)
                    rstd = _rsqrt(nc, sa, consts, vpe[:], 1, f"rs_{t}")
                    nrstd = sa.tile([P, 1], F32, tag="nrs", bufs=4,
                                    name=f"nrs_{t}")
                    nc.gpsimd.tensor_scalar(nrstd, rstd, -1.0, None, OP.mult)
                    # s2 = [mu, std]: psum gets -mu*c + std*d; the rstd scale
                    # later turns it into -mu*rstd*c + d.  std = (var+eps)*rstd
                    s2 = sa.tile([P, 2], F32R, tag="s2", bufs=4,
                                 name=f"s2_{t}")
                    nc.gpsimd.tensor_copy(s2[:, 0:1], mv[:, 0:1])
                    nc.gpsimd.tensor_tensor(s2[:, 1:2], vpe, rstd, OP.mult)
                    # aug rows via two tiny SBUF->SBUF DMA transposes
                    augT = sa.tile([2, P], F32R, tag="augT", bufs=4,
                                   name=f"augT_{t}")
                    nc.gpsimd.dma_start(augT[0:1, :], s2[:, 0:1])
                    nc.gpsimd.dma_start(augT[1:2, :], s2[:, 1:2])
                    return rstd, nrstd, augT

                def transpose_x(t, x_t):
                    """PE transposes + one ACT copy -> xT (tag A)."""
                    xT = sa.tile([P, NC, P], F32R, tag="xT", bufs=2,
                                 name=f"xT_{t}")
                    tp = pa.tile([P, NC * P], F32, tag="A", bufs=2,
                                 name=f"tp_{t}")
                    for blk in range(NC):
                        nc.tensor.transpose(
                            tp[:, blk * P:(blk + 1) * P].bitcast(F32R),
                            x_t[:, blk * P:(blk + 1) * P], ident_r)
                    nc.scalar.copy(xT[:].rearrange("p c s -> p (c s)"), tp)
                    return xT

                x_cur = load_x(0)
                x_nxt = load_x(1)
                load_weights()
                pre = stats_chain(0, x_cur)
                xT_cur = transpose_x(0, x_cur)
                for t in range(NT):
                    rstd, nrstd, augT = pre
                    xT = xT_cur

                    # paired-bank psums: q/v on tag A, k on tag B
                    q_ps = pa.tile([P, HID], F32, tag="A", bufs=2,
                                   name=f"q_ps{t}")
                    v_ps = pa.tile([P, HID], F32, tag="A", bufs=2,
                                   name=f"v_ps{t}")
                    k_ps = pa.tile([P, HID], F32, tag="B", bufs=1,
                                   name=f"k_ps{t}")
                    # k+q first so their psum banks release early; v after
                    for c in range(NC):
                        st = (c == 0)
                        for half in range(2):
                            sl = slice(half * 512, (half + 1) * 512)
                            nc.tensor.matmul(k_ps[:, sl], xT[:, c, :],
                                             w_sb["wk"][:, c, sl],
                                             start=st, stop=(c == NC - 1))
                            nc.tensor.matmul(q_ps[:, sl], xT[:, c, :],
                                             w_sb["wq"][:, c, sl],
                                             start=st, stop=False)
                    # q aug: += mu*(-c) + std*d  (K=2)
                    for half in range(2):
                        sl = slice(half * 512, (half + 1) * 512)
                        nc.tensor.matmul(q_ps[:, sl], augT, cdq_sb[:, sl],
                                         start=False, stop=True)
                    for c in range(NC):
                        for half in range(2):
                            sl = slice(half * 512, (half + 1) * 512)
                            nc.tensor.matmul(v_ps[:, sl], xT[:, c, :],
                                             w_sb["wv"][:, c, sl],
                                             start=(c == 0),
                                             stop=(c == NC - 1))

                    # lookahead: next tile's stats run on DVE/Pool while this
                    # tile's projections stream on PE
                    if t + 1 < NT:
                        pre = stats_chain(t + 1, x_nxt)

                    # ---- elu: full-width PSUM readers first ----
                    raw_k = sa.tile([P, NH, HD], F32, tag="rawk", bufs=2,
                                    name=f"rawk{t}")
                    raw_q = sa.tile([P, NH, HD], F32, tag="rawq", bufs=2,
                                    name=f"rawq{t}")
                    v_sb = sa.tile([P, NH, HD], F32R, tag="vsb", bufs=2,
                                   name=f"v_sb{t}")
                    if bk_nz:
                        xb = sa.tile([P, HID], F32, tag="kxb", bufs=2,
                                     name=f"kxb{t}")
                        nc.vector.tensor_tensor(xb, k_ps, brep["bk"], OP.add)
                        ksrc = xb[:]
                    else:
                        ksrc = k_ps[:]
                    m = sa.tile([P, HID], F32, tag="km", bufs=2,
                                name=f"km{t}")
                    nc.vector.tensor_scalar(m, ksrc, 0.0, 1.0,
                                            OP.max, OP.subtract)
                    r = sa.tile([P, HID], F32, tag="kr", bufs=2, name=f"kr{t}")
                    nc.scalar.activation(r, ksrc, AF.Relu, scale=-1.0)
                    mq = sa.tile([P, HID], F32, tag="qm", bufs=2,
                                 name=f"qm{t}")
                    nc.vector.tensor_scalar(mq, q_ps, rstd, 0.0,
                                            OP.mult, OP.max)
                    rq = sa.tile([P, HID], F32, tag="qr", bufs=2,
                                 name=f"qr{t}")
                    nc.scalar.activation(rq, q_ps, AF.Relu,
                                         scale=nrstd[:, 0:1])

                    # PE: deferred kv + eq-transpose of the previous tile
                    flush_deferred()

                    # v -> SBUF (ACT, before the xT copy in queue order)
                    vdst = v_sb[:].rearrange("p h d -> p (h d)")
                    if bv_nz:
                        nc.vector.tensor_tensor(vdst, v_ps, brep["bv"], OP.add)
                    else:
                        nc.scalar.copy(vdst, v_ps)

                    # next tile's x transposes (tag-A bank just released)
                    if t + 1 < NT:
                        xT_cur = transpose_x(t + 1, x_nxt)
                        x_cur = x_nxt
                        if t + 2 < NT:
                            x_nxt = load_x(t + 2)

                    # ---- SBUF tail: exp, combine, squares ----
                    e = sa.tile([P, HID], F32, tag="ke", bufs=2, name=f"ke{t}")
                    nc.scalar.activation(e, r, AF.Exp, scale=-1.0)
                    nc.gpsimd.tensor_tensor(
                        raw_k[:].rearrange("p h d -> p (h d)"), m, e, OP.add)
                    eq_ = sa.tile([P, HID], F32, tag="qe", bufs=2,
                                  name=f"qe{t}")
                    nc.scalar.activation(eq_, rq, AF.Exp, scale=-1.0)
                    rawh = raw_q[:].rearrange("p h d -> p (h d)")
                    nc.gpsimd.tensor_tensor(rawh, mq, eq_, OP.add)
                    nc.gpsimd.tensor_scalar(rawh, rawh, 1.0, None, OP.subtract)

                    # ---- per-head l2 norms (one batched rsqrt) + scale ----
                    sq = sa.tile([P, NH, HD], F32, tag="sq", bufs=2,
                                 name=f"sq{t}")
                    ss = sa.tile([P, 2, NH], F32, tag="ss", bufs=4,
                                 name=f"ss{t}")
                    nc.gpsimd.tensor_tensor(
                        sq[:].rearrange("p h d -> p (h d)"),
                        raw_k[:].rearrange("p h d -> p (h d)"),
                        raw_k[:].rearrange("p h d -> p (h d)"), OP.mult)
                    nc.vector.tensor_reduce(ss[:, 0, :], sq,
                                            mybir.AxisListType.X, OP.add)
                    sq2 = sa.tile([P, NH, HD], F32, tag="sq", bufs=2,
                                  name=f"sq2{t}")
                    nc.gpsimd.tensor_tensor(
                        sq2[:].rearrange("p h d -> p (h d)"),
                        raw_q[:].rearrange("p h d -> p (h d)"),
                        raw_q[:].rearrange("p h d -> p (h d)"), OP.mult)
                    nc.vector.tensor_reduce(ss[:, 1, :], sq2,
                                            mybir.AxisListType.X, OP.add)
                    rn = _rsqrt(nc, sa, consts,
                                ss[:].rearrange("p a h -> p (a h)"), 2 * NH,
                                f"rn{t}")
                    ek = sa.tile([P, NH, HD], F32R, tag="ek", bufs=2,
                                 name=f"ek{t}")
                    eqs = sa.tile([P, NH, HD], F32R, tag="eq", bufs=2,
                                  name=f"eqs{t}")
                    nc.vector.tensor_tensor(
                        ek, raw_k,
                        rn[:, 0:NH][:, :, None].to_broadcast((P, NH, HD)),
                        OP.mult)
                    nc.vector.tensor_tensor(
                        eqs, raw_q,
                        rn[:, NH:2 * NH][:, :, None].to_broadcast((P, NH, HD)),
                        OP.mult)
                    if debug and t == 0:
                        nc.sync.dma_start(
                            dbg["ek0"].ap(),
                            ek[:].rearrange("p h d -> p (h d)").bitcast(F32))
                        nc.sync.dma_start(
                            dbg["v0"].ap(),
                            v_sb[:].rearrange("p h d -> p (h d)").bitcast(F32))
                        nc.sync.dma_start(
                            dbg["eq0"].ap(),
                            eqs[:].rearrange("p h d -> p (h d)").bitcast(F32))

                    deferred[0] = (t, ek, v_sb, eqs)
                flush_deferred()

                # kv state -> SBUF bf16, packed 2 heads per 128 partitions
                kvv = kv_ps[:].rearrange("p (a r d) -> p a r d", r=2, d=HD)
                kvb = kv_sb[:].rearrange("p (a d) -> p a d", d=HD)
                nc.vector.tensor_copy(kvb[0:HD], kvv[:, :, 0, :])
                nc.vector.tensor_copy(kvb[HD:P], kvv[:, :, 1, :])
                if debug:
                    kvstage = sa.tile([HD, NH * HD], F32, name="kvstage")
                    nc.vector.tensor_copy(kvstage, kv_ps)
                    nc.sync.dma_start(dbg["kv"].ap(), kvstage)

            # ---------------- pass B: ctx -> out ---------------------------
            with tc.tile_pool(name="sbufB", bufs=1) as sb, \
                 tc.tile_pool(name="psumB", bufs=1, space="PSUM") as pb:
                wo_sb = wpool.tile([P, NC, HID], F32R, tag="w_wq",
                                   name="wo_sb")
                nc.sync.dma_start(
                    wo_sb,
                    wot_d.ap().rearrange("(c p) j -> p c j", p=P).bitcast(F32R))

                deferred_o = [None]

                def flush_o():
                    d = deferred_o[0]
                    if d is None:
                        return
                    chd, ctxT = d
                    deferred_o[0] = None
                    for tl in range(CHUNK):
                        t = chd * CHUNK + tl
                        x_t2 = sb.tile([P, HID], F32, tag="x2", bufs=3,
                                       name=f"x2_{t}")
                        nc.sync.dma_start(
                            x_t2, x_d.ap()[t * P:(t + 1) * P, :])
                        if bo_nz:
                            xb2 = sb.tile([P, HID], F32, tag="xb2", bufs=2,
                                          name=f"xb2_{t}")
                            nc.gpsimd.tensor_tensor(xb2, x_t2, brep["bo"],
                                                    OP.add)
                            res = xb2
                        else:
                            res = x_t2
                        out_sb = sb.tile([P, HID], F32, tag="osb", bufs=2,
                                         name=f"out_{t}")
                        o_ps = pb.tile([P, HID], F32, tag="oh", bufs=2,
                                       name=f"o_ps{t}")
                        for half in range(2):
                            sl = slice(half * 512, (half + 1) * 512)
                            for c in range(NC):
                                nc.tensor.matmul(
                                    o_ps[:, sl],
                                    ctxT[:, c, tl * P:(tl + 1) * P],
                                    wo_sb[:, c, sl],
                                    start=(c == 0), stop=(c == NC - 1))
                        nc.vector.tensor_tensor(out_sb, o_ps, res, OP.add)
                        nc.gpsimd.dma_start(
                            out_d.ap()[t * P:(t + 1) * P, :], out_sb)

                for ch in range(NCHUNKS):
                    eqt_ld = sb.tile([P, NC, CHUNK, P], BF16, tag="eqld",
                                     bufs=2, name=f"eqld{ch}")
                    for tl in range(CHUNK):
                        t = ch * CHUNK + tl
                        nc.sync.dma_start(
                            eqt_ld[:, :, tl, :],
                            eqt_d.ap()[t].rearrange("p (c s) -> p c s", s=P))

                    # ctx^T per head pair
                    ctxT = sb.tile([P, NC, CHUNK * P], F32R, tag="ctxT",
                                   bufs=2, name=f"ctxT{ch}")
                    for jt in range(NC):
                        cs = slice(jt * HD, (jt + 1) * HD)
                        c_pse = pb.tile([HD, CHUNK * P], F32, tag="ctxe",
                                        bufs=2, name=f"c_pse{ch}_{jt}")
                        c_pso = pb.tile([HD, CHUNK * P], F32, tag="ctxo",
                                        bufs=2, name=f"c_pso{ch}_{jt}")
                        nc.tensor.matmul(c_pse, kv_sb[0:HD, cs],
                                         eqt_ld[0:HD, jt, :, :],
                                         start=True, stop=True)
                        nc.tensor.matmul(c_pso, kv_sb[HD:P, cs],
                                         eqt_ld[HD:P, jt, :, :],
                                         start=True, stop=True)
                        nc.scalar.copy(ctxT[0:HD, jt, :], c_pse)
                        nc.vector.tensor_copy(ctxT[HD:P, jt, :], c_pso)

                    flush_o()
                    deferred_o[0] = (ch, ctxT)
                flush_o()

    nc.compile()
    return nc


_RUNNER = {}


def _get_runner(loop_n=1, flags=(False, False, False)):
    key = (loop_n, flags)
    if key in _RUNNER:
        return _RUNNER[key]

    import jax
    from jax.sharding import Mesh, PartitionSpec
    from jax.experimental.shard_map import shard_map
    from concourse.bass2jax import _bass_exec_p, install_neuronx_cc_hook

    install_neuronx_cc_hook()
    nc = build_nc(loop_n=loop_n, bk_nz=flags[0], bv_nz=flags[1],
                  bo_nz=flags[2])

    in_names = []
    out_names = []
    out_avals = []
    for alloc in nc.m.functions[0].allocations:
        if not isinstance(alloc, mybir.MemoryLocationSet):
            continue
        name = alloc.memorylocations[0].name
        if alloc.kind == "ExternalInput":
            in_names.append(name)
        elif alloc.kind == "ExternalOutput":
            out_names.append(name)
            out_avals.append(
                jax.core.ShapedArray(tuple(alloc.tensor_shape),
                                     mybir.dt.np(alloc.dtype)))
    n_params = len(in_names)
    all_in_names = in_names + out_names

    def _body(*args):
        outs = _bass_exec_p.bind(
            *args,
            out_avals=tuple(out_avals),
            in_names=tuple(all_in_names),
            out_names=tuple(out_names),
            lowering_input_output_aliases=(),
            sim_require_finite=True,
            sim_require_nnan=True,
            nc=nc,
        )
        return tuple(outs)

    devices = jax.devices()[:B]
    mesh = Mesh(np.asarray(devices), ("core",))
    n_outs = len(out_names)
    fn = jax.jit(
        shard_map(
            _body, mesh=mesh,
            in_specs=(PartitionSpec("core"),) * (n_params + n_outs),
            out_specs=(PartitionSpec("core"),) * n_outs,
            check_rep=False,
        ),
        keep_unused=True,
    )
    _RUNNER[key] = (fn, in_names, out_names, out_avals)
    return _RUNNER[key]


def prep_inputs(input_tensor, attention_mask, ln_gamma, ln_beta,
                Wq, bq, Wk, bk, Wv, bv, Wo, bo):
    """Host-side static prep: transpose weights, fold LN into q projection."""
    f = np.float32
    x = np.ascontiguousarray(np.asarray(input_tensor, f))
    g = np.asarray(ln_gamma, f)
    be = np.asarray(ln_beta, f)
    Wq = np.asarray(Wq, f); Wk = np.asarray(Wk, f)
    Wv = np.asarray(Wv, f); Wo = np.asarray(Wo, f)
    wg = np.ascontiguousarray((Wq * g[None, :]).T)          # [i, j]
    c = wg.sum(axis=0)                                      # [j]
    d = (np.asarray(bq, f) + be @ Wq.T).astype(f)           # [j]
    cdq = np.stack([-c, d], axis=0).astype(f)               # [2, j]
    wkt = np.ascontiguousarray(Wk.T)
    wvt = np.ascontiguousarray(Wv.T)
    wot = np.ascontiguousarray(Wo.T * np.float32(1.0 / np.sqrt(HD)))
    per_core = {
        "wqt": wg, "wkt": wkt, "wvt": wvt, "wot": wot,
        "cdq": cdq,
        "bk": np.asarray(bk, f).reshape(1, HID),
        "bv": np.asarray(bv, f).reshape(1, HID),
        "bo": np.asarray(bo, f).reshape(1, HID),
    }
    return x, per_core


def kernel(**inputs) -> np.ndarray:
    x, per_core = prep_inputs(**inputs)
    flags = (bool(np.any(per_core["bk"])), bool(np.any(per_core["bv"])),
             bool(np.any(per_core["bo"])))
    fn, in_names, out_names, out_avals = _get_runner(1, flags)

    concat_in = []
    for name in in_names:
        if name == "x":
            concat_in.append(x.reshape(B * S, HID))
        else:
            concat_in.append(np.concatenate([per_core[name]] * B, axis=0))
    concat_zeros = [
        np.zeros((B * av.shape[0], *av.shape[1:]), av.dtype) for av in out_avals
    ]
    out_arrs = fn(*concat_in, *concat_zeros)
    out = np.asarray(out_arrs[out_names.index("out")])
    return out.reshape(B, S, HID)
